# revision 1
# baseline (speedup 1.0000x reference)
"""Trainium2 Bass kernel for nn_MultiHeadAttention (B=32, S=1024, E=64, H=8, D=64).

Strategy (per core; batch-parallel over 8 cores, 4 batches each):
  - Host-side numpy prep: permute w_qkv columns into per-head Q/K/V blocks,
    transpose x to xT (head-dim on partitions), duplicate operands across
    both 64-partition halves so pairs of K=64 matmuls run as PE row-tile
    pairs. V is pre-scaled by 1/8 (the post-softmax scale).
  - On chip, everything stays in "transposed" layouts so no PE transposes
    are needed anywhere:
      qT/kT:   [2 heads * 64 d, 1024 nq]  (4 tiles per batch)
      V:       [128 nk-chunk, 8 heads * (64 v | 1)]  ones col => rowsums
      E^T:     [128 nk, 2 heads * 512 nq] per chunk -> exp (ScalarE) -> P^T
      P^T @ [V|1]: accumulates [65, 512] per head: rows 0..63 = (P V)/8,
                   row 64 = rowsum.  softmax normalization = multiply by
                   broadcast reciprocal of row 64 (no max subtraction:
                   energies are |E| < ~60, exp stays in fp32 range; softmax
                   is shift invariant so this matches the reference).
      proj:    per-head K=64 matmuls accumulate yT [64 e, nq]; output is
               returned transposed and fixed up on host.
  - The whole kernel is emitted as one software-pipelined stream: each
    attention unit's eT matmuls run two chunks ahead of the PV matmuls
    (pv queue with lag 2), so the exp stream on ScalarE — the bottleneck
    engine — never waits on the in-order PE queue.  QKV matmuls for the
    next batch, the normalize multiplies, and the output projections are
    injected into fixed chunk slots of later units so they never stall
    the eT->exp chain.
  - Softmax normalization runs per head-pair as soon as a unit finishes:
    rowsum rows are gathered by SBUF->SBUF DMAs into a [2, 512] tile,
    a 2-op Newton reciprocal on DVE, a bf16 downcast, DRAM hop, and two
    partition-broadcast DMAs.  The normalize multiplies fire mult_depth
    units later, projections one unit after that, and yT stores are split
    by half across two DMA queues to shrink the endgame tail.
"""

import os
import sys

import numpy as np

_TRN_REPO = "/opt/trn_rl_repo"
if _TRN_REPO not in sys.path:
    sys.path.insert(0, _TRN_REPO)

B, S, E, H, D = 32, 1024, 64, 8, 64
HID = H * D  # 512
N_CORES = 8
NQH = 512  # nq half processed per psum tile


def build_nc(bb=B // N_CORES, dt_e="f32r", dt_pv="f32r", dt_qkv="f32r", dt_proj="f32r",
             reps=1, lag=2, mult_depth=2, proj_depth=3):
    """Build the per-core Bass kernel. bb = batches per core."""
    import concourse.bass as bass
    import concourse.mybir as mybir
    import concourse.tile as tile
    from concourse import bacc
    from contextlib import ExitStack

    f32 = mybir.dt.float32
    f32r = mybir.dt.float32r
    bf16 = mybir.dt.bfloat16
    Exp = mybir.ActivationFunctionType.Exp

    def dt_of(key):
        return f32r if key == "f32r" else f32

    dte, dtpv, dtqkv, dtproj = dt_of(dt_e), dt_of(dt_pv), dt_of(dt_qkv), dt_of(dt_proj)

    nc = bacc.Bacc(None, target_bir_lowering=False)

    # ---- DRAM I/O (host-prepped layouts) ----
    xT_d = nc.dram_tensor("xT", [bb, 128, S], dtqkv, kind="ExternalInput")
    wq_d = nc.dram_tensor("wq", [128, HID], dtqkv, kind="ExternalInput")
    wk_d = nc.dram_tensor("wk", [128, HID], dtqkv, kind="ExternalInput")
    wv_d = nc.dram_tensor("wv", [128, HID], dtqkv, kind="ExternalInput")  # pre /8
    bqk_d = nc.dram_tensor("bqk", [128, 8], f32, kind="ExternalInput")
    bv_d = nc.dram_tensor("bv", [HID], f32, kind="ExternalInput")  # pre /8
    wp_d = nc.dram_tensor("wp", [64, H, E], dtproj, kind="ExternalInput")
    bp_d = nc.dram_tensor("bp", [E, 1], f32, kind="ExternalInput")
    yT_d = nc.dram_tensor("yT", [bb, E, S], f32, kind="ExternalOutput")

    with tile.TileContext(nc) as tc, ExitStack() as ctx:
        wpool = ctx.enter_context(tc.tile_pool(name="weights", bufs=1))
        qkpool = ctx.enter_context(tc.tile_pool(name="qk", bufs=2))
        vpool = ctx.enter_context(tc.tile_pool(name="v", bufs=2))
        ptpool = ctx.enter_context(tc.tile_pool(name="pt", bufs=5))
        ovpool = ctx.enter_context(tc.tile_pool(name="ov", bufs=16))
        rbpool = ctx.enter_context(tc.tile_pool(name="rb", bufs=10))
        miscpool = ctx.enter_context(tc.tile_pool(name="misc", bufs=2))
        psum_e = ctx.enter_context(tc.tile_pool(name="psum_e", bufs=2, space="PSUM"))
        psum_s = ctx.enter_context(tc.tile_pool(name="psum_s", bufs=4, space="PSUM"))
        drampool = ctx.enter_context(tc.tile_pool(name="dram", bufs=4, space="DRAM"))

        # ---- weights/biases ----
        # Startup is gated by the global DMA transfer stage, so the bytes the
        # first eT->exp chain needs (bqk, wq/wk cols 0:256, xT cols 0:512,
        # then wv) are issued first on the two hardware DGE queues; the rest
        # rides the gpsimd software DGE, whose Pool-engine serialization and
        # ~1us/DMA overhead only bulk non-critical loads can tolerate.
        wq_sb = wpool.tile([128, HID], dtqkv)
        wk_sb = wpool.tile([128, HID], dtqkv)
        wv_sb = wpool.tile([128, HID], dtqkv)
        bqk_sb = wpool.tile([128, 8], f32)
        nc.sync.dma_start(out=bqk_sb, in_=bqk_d[:, :])
        nc.scalar.dma_start(out=wq_sb[:, 0:256], in_=wq_d[:, 0:256])
        nc.sync.dma_start(out=wk_sb[:, 0:256], in_=wk_d[:, 0:256])

        def load_weight_tail():
            nc.scalar.dma_start(out=wv_sb[:, 0:256], in_=wv_d[:, 0:256])
            nc.sync.dma_start(out=wv_sb[:, 256:512], in_=wv_d[:, 256:512])
            nc.scalar.dma_start(
                out=bv_sb, in_=bv_d[:].unsqueeze(0).partition_broadcast(128)
            )
            nc.sync.dma_start(out=wq_sb[:, 256:512], in_=wq_d[:, 256:512])
            nc.scalar.dma_start(out=wk_sb[:, 256:512], in_=wk_d[:, 256:512])
            nc.sync.dma_start(out=wp_sb, in_=wp_d[:, :, :])
            nc.scalar.dma_start(out=bp_sb, in_=bp_d[:, :])

        bv_sb = wpool.tile([128, HID], f32)
        wp_sb = wpool.tile([64, H, E], dtproj)
        bp_sb = wpool.tile([E, 1], f32)
        ones_sb = wpool.tile([128, H], f32)
        nc.vector.memset(ones_sb, 1.0)
        ones64 = wpool.tile([1, 64], f32, name="ones64")
        nc.vector.memset(ones64, 1.0)

        def alloc_batch(bi, b, first=False):
            xT_sb = qkpool.tile([128, S], dtqkv, tag="xT", name=f"xT_{b}")
            # 4-way column split across two DGE queues: lands sooner, and
            # early qkv groups only depend on the columns they read.  The
            # first batch avoids the software DGE entirely.
            q2 = nc.scalar if first else nc.gpsimd
            nc.sync.dma_start(out=xT_sb[:, 0:256], in_=xT_d[bi][:, 0:256])
            q2.dma_start(out=xT_sb[:, 256:512], in_=xT_d[bi][:, 256:512])
            nc.sync.dma_start(out=xT_sb[:, 512:768], in_=xT_d[bi][:, 512:768])
            q2.dma_start(out=xT_sb[:, 768:1024], in_=xT_d[bi][:, 768:1024])
            if first:
                load_weight_tail()
            qT = [qkpool.tile([128, S], dte, tag=f"qT{t}", name=f"qT{t}_{b}") for t in range(4)]
            kT = [qkpool.tile([128, S], dte, tag=f"kT{t}", name=f"kT{t}_{b}") for t in range(4)]
            v_nat = [vpool.tile([128, H * 65], dtpv, tag=f"v{c}", name=f"v{c}_{b}") for c in range(8)]
            return dict(bi=bi, b=b, xT=xT_sb, qT=qT, kT=kT, v=v_nat,
                        ov={}, oT={}, recB={}, pt={})

        def emit_qk_pair(st, qki, tp, halves=(0, 1)):
            w_sb = (wq_sb, wk_sb)[qki]
            dst = (st["qT"], st["kT"])[qki]
            xT_sb, b = st["xT"], st["b"]
            for half in halves:
                nq = slice(half * NQH, (half + 1) * NQH)
                ps_e = psum_s.tile([128, NQH], f32, tag="small", name=f"psqkv_e{b}_{qki}{tp}{half}")
                ps_o = psum_s.tile([128, NQH], f32, tag="small", name=f"psqkv_o{b}_{qki}{tp}{half}")
                nc.tensor.matmul(ps_e, w_sb[0:64, 128 * tp : 128 * (tp + 1)], xT_sb[0:64, nq])
                nc.tensor.matmul(ps_o, w_sb[64:128, 128 * (tp + 1) : 128 * (tp + 2)], xT_sb[64:128, nq])
                nc.vector.tensor_scalar_add(
                    dst[tp][:, nq], ps_e, bqk_sb[:, qki * 4 + tp : qki * 4 + tp + 1]
                )
                nc.vector.tensor_scalar_add(
                    dst[tp + 1][:, nq], ps_o, bqk_sb[:, qki * 4 + tp + 1 : qki * 4 + tp + 2]
                )

        def emit_v_pair(st, cp):
            xT_sb, v_nat, b = st["xT"], st["v"], st["b"]
            write_ones = True
            ps_e = psum_s.tile([128, HID], f32, tag="small", name=f"psv_e{b}_{cp}")
            ps_o = psum_s.tile([128, HID], f32, tag="small", name=f"psv_o{b}_{cp}")
            nc.tensor.matmul(ps_e, xT_sb[0:64, 128 * cp : 128 * (cp + 1)], wv_sb[0:64, :])
            nc.tensor.matmul(ps_o, xT_sb[64:128, 128 * (cp + 1) : 128 * (cp + 2)], wv_sb[64:128, :])
            for c, pss in ((cp, ps_e), (cp + 1, ps_o)):
                vdst = v_nat[c].rearrange("p (h c65) -> p h c65", c65=65)
                nc.vector.tensor_tensor(
                    vdst[:, :, 0:64],
                    pss.rearrange("p (h d) -> p h d", d=64),
                    bv_sb.rearrange("p (h d) -> p h d", d=64),
                    mybir.AluOpType.add,
                )
                if write_ones:
                    nc.vector.tensor_copy(vdst[:, :, 64], ones_sb)

        # ---- software-pipelined attention emission ----
        pv_q = []  # pending (st, hp, half, c) PV emissions (lag behind eT)

        def emit_eT(st, hp, half, c):
            qT, kT, b = st["qT"], st["kT"], st["b"]
            nq = slice(half * NQH, (half + 1) * NQH)
            eT = psum_e.tile([128, 2 * NQH], f32, tag="eT", name=f"eT_{b}_{hp}_{half}_{c}")
            nc.tensor.matmul(
                eT[:, 0:NQH], kT[hp][0:64, 128 * c : 128 * (c + 1)], qT[hp][0:64, nq]
            )
            nc.tensor.matmul(
                eT[:, NQH : 2 * NQH],
                kT[hp][64:128, 128 * c : 128 * (c + 1)],
                qT[hp][64:128, nq],
            )
            pt = ptpool.tile([128, 2 * NQH], dtpv, tag="pt", name=f"pt_{b}_{hp}_{half}_{c}")
            nc.scalar.activation(pt, eT, Exp)
            st["pt"][(hp, half, c)] = pt
            pv_q.append((st, hp, half, c))

        def pop_pv():
            st, hp, half, c = pv_q.pop(0)
            b, v_nat = st["b"], st["v"]
            if c == 0:
                st["oT"][(hp, half)] = (
                    psum_s.tile([65, NQH], f32, tag="small", name=f"oTe_{b}_{hp}_{half}"),
                    psum_s.tile([65, NQH], f32, tag="small", name=f"oTo_{b}_{hp}_{half}"),
                )
            oT_e, oT_o = st["oT"][(hp, half)]
            pt = st["pt"].pop((hp, half, c))
            nc.tensor.matmul(
                oT_e,
                v_nat[c][:, (2 * hp) * 65 : (2 * hp) * 65 + 65],
                pt[:, 0:NQH],
                start=(c == 0),
                stop=(c == 7),
            )
            nc.tensor.matmul(
                oT_o,
                v_nat[c][:, (2 * hp + 1) * 65 : (2 * hp + 1) * 65 + 65],
                pt[:, NQH : 2 * NQH],
                start=(c == 0),
                stop=(c == 7),
            )
            if c == 7:
                finish_unit(st, hp, half)

        def finish_unit(st, hp, half):
            """ov copies, rowsum gather, and the per-pair reciprocal ->
            broadcast chain (all off the critical eT->exp path)."""
            b = st["b"]
            oT_pair = st["oT"].pop((hp, half))
            # the run's very last pair takes the express path: reciprocal
            # straight off the ov rowsum row (engine partition offset 64 is
            # legal), broadcast later by a K=1 PE matmul — no DMA round trips
            fast = st.get("_last") and half == 1 and hp == 3
            rs_dram = drampool.tile([2, NQH], dtproj, tag="rsd", name=f"rsd_{b}_{hp}_{half}", bufs=4)
            for par, oT in enumerate(oT_pair):
                h = 2 * hp + par
                t = ovpool.tile([65, NQH], dtproj, tag="ov", name=f"ov_{b}_{h}_{half}")
                nc.vector.tensor_copy(t, oT)
                st["ov"][(h, half)] = t
                # rowsum row -> DRAM (engine APs can't start at partition 64,
                # and SBUF->SBUF DMA misbehaves on hw, so hop through DRAM)
                nc.sync.dma_start(out=rs_dram[par : par + 1, :], in_=t[64:65, :])
            if fast:
                # endgame pair: per-head [1, 512] reciprocal tiles feed K=1
                # broadcast matmuls (moving operands must start at partition 0)
                for par in range(2):
                    h = 2 * hp + par
                    xin = miscpool.tile([1, NQH], dtproj, tag=f"xi{par}", name=f"xi_{b}_{h}", bufs=1)
                    nc.sync.dma_start(out=xin, in_=rs_dram[par : par + 1, :])
                    xr = miscpool.tile([1, NQH], f32, tag=f"xr{par}", name=f"xr_{b}_{h}", bufs=1)
                    xscr = miscpool.tile([1, NQH], f32, tag="xscr", name=f"xscr_{b}_{h}", bufs=1)
                    nc.vector.reciprocal_approx_accurate(xr, xin.bitcast(f32), xscr)
                    st.setdefault("xr", {})[(h, half)] = xr
                return
            rs = miscpool.tile([2, NQH], dtproj, tag="rsp", name=f"rs_{b}_{hp}_{half}", bufs=2)
            nc.sync.dma_start(out=rs, in_=rs_dram[:, :])
            rcp = miscpool.tile([2, NQH], f32, tag="rcp", name=f"rcp_{b}_{hp}_{half}", bufs=2)
            rscr = miscpool.tile([2, NQH], f32, tag="rscr", name=f"rscr_{b}_{hp}_{half}", bufs=1)
            nc.vector.reciprocal_approx_accurate(rcp, rs.bitcast(f32), rscr)
            rcp_b = miscpool.tile([2, NQH], bf16, tag="rcpb", name=f"rcpb_{b}_{hp}_{half}", bufs=2)
            nc.vector.tensor_copy(rcp_b, rcp)
            rcp_dram = drampool.tile([2, NQH], bf16, tag="rcpd", name=f"rcpd_{b}_{hp}_{half}", bufs=8)
            nc.sync.dma_start(out=rcp_dram, in_=rcp_b)
            for par in range(2):
                h = 2 * hp + par
                recB = rbpool.tile([64, NQH], bf16, tag="recB", name=f"recB_{b}_{h}_{half}")
                nc.sync.dma_start(
                    out=recB, in_=rcp_dram[par : par + 1, :].partition_broadcast(64)
                )
                st["recB"][(h, half)] = recB

        def emit_mults(st, half, heads=range(H)):
            ov, recB = st["ov"], st["recB"]
            for h in heads:
                nc.vector.tensor_tensor(
                    ov[(h, half)][0:64, :],
                    ov[(h, half)][0:64, :],
                    recB.pop((h, half)),
                    mybir.AluOpType.mult,
                )

        def emit_proj(st, half, hs=(0, H), tail=False):
            ov, b, bi = st["ov"], st["b"], st["bi"]
            if "yT" not in st:
                st["yT"] = miscpool.tile([E, S], f32, tag="yT", name=f"yTsb_{b}", bufs=2)
            yT_sb = st["yT"]
            nq = slice(half * NQH, (half + 1) * NQH)
            if hs[0] == 0:
                st.setdefault("yT_ps", {})[half] = psum_s.tile(
                    [E, NQH], f32, tag="small", name=f"yTps_{b}_{half}"
                )
            yT_ps = st["yT_ps"][half]
            for h in range(*hs):
                if tail:
                    # endgame: interleave mult h+1 (DVE) with proj h (PE)
                    nc.vector.tensor_tensor(
                        ov[(h, half)][0:64, :],
                        ov[(h, half)][0:64, :],
                        st["recB"].pop((h, half)),
                        mybir.AluOpType.mult,
                    )
                nc.tensor.matmul(
                    yT_ps,
                    wp_sb[:, h, :],
                    ov.pop((h, half))[0:64, :],
                    start=(h == 0),
                    stop=(h == H - 1),
                )
            if hs[1] < H:
                return
            nc.vector.tensor_scalar_add(yT_sb[:, nq], yT_ps, bp_sb)
            # split stores by half and across queues: the final store is small
            dma_a = nc.scalar if tail else nc.sync
            dma_b = nc.scalar if tail else nc.gpsimd
            o0 = half * NQH
            dma_a.dma_start(out=yT_d[bi][:, o0 : o0 + 256], in_=yT_sb[:, o0 : o0 + 256])
            dma_b.dma_start(out=yT_d[bi][:, o0 + 256 : o0 + NQH], in_=yT_sb[:, o0 + 256 : o0 + NQH])

        # ---- schedule ----
        batches = [(rep, bi) for rep in range(reps) for bi in range(bb)]
        sts = {0: alloc_batch(batches[0][1], batches[0][0] * 1000 + batches[0][1], first=True)}
        emit_qk_pair(sts[0], 0, 0, halves=(0,))
        emit_qk_pair(sts[0], 1, 0, halves=(0,))

        mult_due = []  # (unit_idx_done, st, half)
        proj_due = []
        nunits = 0

        def fillers(i, st, hp, half, c, last):
            """Extra work injected at chunk slot c of unit (hp, half)."""
            prologue = i == 0 and half == 0 and hp == 0
            if prologue:
                # spread the remaining first-batch qkv through unit 0's slots:
                # V pairs for chunks 4-7 and the kT/qT half-1 tiles come later
                # (their xT columns arrive later), projection-weight tiles last
                if c == 0:
                    emit_v_pair(st, 0)
                    emit_v_pair(st, 2)
                elif c == 1:
                    emit_qk_pair(st, 1, 0, halves=(1,))
                elif c == 2:
                    emit_v_pair(st, 4)
                    emit_v_pair(st, 6)
                elif c == 3:
                    emit_qk_pair(st, 0, 0, halves=(1,))
                elif c == 4 and i + 1 < len(batches):
                    rep, bi = batches[i + 1]
                    sts[i + 1] = alloc_batch(bi, rep * 1000 + bi)
                elif c == 5:
                    emit_qk_pair(st, 0, 2)
                elif c == 6:
                    emit_qk_pair(st, 1, 2)
                return
            if c == 2 and half == 0 and hp == 0 and i + 1 < len(batches):
                rep, bi = batches[i + 1]
                sts[i + 1] = alloc_batch(bi, rep * 1000 + bi)
            if half == 1 and i + 1 < len(batches):
                # next batch's qkv, spread in 2-matmul micro-bursts so the
                # eT->exp cadence never hiccups
                nxt = sts[i + 1]
                if hp == 0:
                    if c == 2:
                        emit_qk_pair(nxt, 0, 0, halves=(0,))
                    elif c == 4:
                        emit_qk_pair(nxt, 0, 0, halves=(1,))
                elif hp == 1:
                    if c == 2:
                        emit_v_pair(nxt, 0)
                    elif c == 4:
                        emit_v_pair(nxt, 2)
                elif hp == 2:
                    if c == 2:
                        emit_v_pair(nxt, 4)
                    elif c == 4:
                        emit_v_pair(nxt, 6)
                else:
                    if c == 1:
                        emit_qk_pair(nxt, 1, 0, halves=(0,))
                    elif c == 2:
                        emit_qk_pair(nxt, 1, 0, halves=(1,))
                    elif c == 3:
                        emit_qk_pair(nxt, 0, 2, halves=(0,))
                    elif c == 4:
                        emit_qk_pair(nxt, 0, 2, halves=(1,))
                    elif c == 6:
                        emit_qk_pair(nxt, 1, 2, halves=(0,))
                    elif c == 7:
                        emit_qk_pair(nxt, 1, 2, halves=(1,))
            if last and half == 1:
                if hp == 2 and c == 3:
                    emit_mults(st, 1, heads=(0, 1))
                elif hp == 3 and c == 1:
                    emit_mults(st, 1, heads=(2, 3))
                elif hp == 3 and c == 3:
                    emit_proj(st, 1, hs=(0, 4))
            if c == 5:
                while mult_due and (nunits >= mult_due[0][0] + mult_depth or (last and len(mult_due) > 1)):
                    _, st_m, half_m = mult_due.pop(0)
                    if last and half_m == 1 and st_m is st:
                        continue  # final half handled by the endgame path
                    emit_mults(st_m, half_m)
                    proj_due.append((nunits, st_m, half_m))
            if c in (6, 7):
                hs = (0, 4) if c == 6 else (4, H)
                if proj_due and (nunits >= proj_due[0][0] + (proj_depth - mult_depth) or (last and len(proj_due) > 1)):
                    _, st_p, half_p = proj_due[0]
                    emit_proj(st_p, half_p, hs=hs)
                    if c == 7:
                        proj_due.pop(0)

        for i in range(len(batches)):
            st = sts.pop(i)
            last = i + 1 >= len(batches)
            st["_last"] = last
            for half in (0, 1):
                for hp in range(4):
                    for c in range(8):
                        emit_eT(st, hp, half, c)
                        fillers(i, st, hp, half, c, last)
                        while len(pv_q) > lag:
                            pop_pv()
                    nunits += 1
                mult_due.append((nunits, st, half))

        while pv_q:
            pop_pv()
        while proj_due:
            _, st_p, half_p = proj_due.pop(0)
            emit_proj(st_p, half_p)
        while mult_due:
            _, st_m, half_m = mult_due.pop(0)
            if (st_m.get("_last") and half_m == 1):
                # endgame: mults for pairs 0/1 and proj h0-3 already emitted
                emit_mults(st_m, 1, heads=(4, 5))
                emit_proj(st_m, 1, hs=(4, 6))
                # express broadcast for pair 3: K=1 matmul into a free eT tile
                bc = psum_e.tile([128, 2 * NQH], f32, tag="eT", name="bc_p3")
                for par in range(2):
                    h = 6 + par
                    rb = bc[0:64, par * NQH : (par + 1) * NQH]
                    nc.tensor.matmul(rb, ones64[0:1, :], st_m["xr"][(h, 1)])
                    st_m["recB"][(h, 1)] = rb
                emit_mults(st_m, 1, heads=(6, 7))
                emit_proj(st_m, 1, hs=(6, H))
            else:
                emit_proj(st_m, half_m, tail=True)

    nc.compile()
    return nc


def _round_f32r(a):
    """Round fp32 to fp32r (11-bit mantissa, RNE) so DMA'd operands are
    pre-rounded as the BIR verifier requires for fp32r matmul consumers."""
    u = np.ascontiguousarray(a, np.float32).view(np.uint32)
    r = (u.astype(np.uint64) + 0x7FF + ((u >> 12) & 1)).astype(np.uint32) & np.uint32(
        0xFFFFF000
    )
    return r.view(np.float32)


def prep_inputs(x, w_qkv, b_qkv, w_proj, b_proj, bb=B // N_CORES, n_cores=N_CORES,
                variant=("f32r", "f32r", "f32r", "f32r")):
    """Host-side prep: permute/duplicate weights, transpose x, shard over cores."""
    x = np.asarray(x, np.float32)
    w_qkv = np.asarray(w_qkv, np.float32)
    b_qkv = np.asarray(b_qkv, np.float32)
    w_proj = np.asarray(w_proj, np.float32)
    b_proj = np.asarray(b_proj, np.float32)

    W = w_qkv.reshape(E, H, D, 3)
    wq = np.ascontiguousarray(W[..., 0].reshape(E, HID))
    wk = np.ascontiguousarray(W[..., 1].reshape(E, HID))
    wv = np.ascontiguousarray(W[..., 2].reshape(E, HID)) / 8.0
    wq_dup = np.concatenate([wq, wq], 0)  # [128, 512]
    wk_dup = np.concatenate([wk, wk], 0)
    wv_dup = np.concatenate([wv, wv], 0)

    Bq = b_qkv.reshape(H, D, 3)
    bq = Bq[..., 0].reshape(HID)
    bk = Bq[..., 1].reshape(HID)
    bv = Bq[..., 2].reshape(HID) / 8.0
    # bqk[p, qki*4 + t] = bias for qT/kT tile t partition p
    bqk = np.zeros((128, 8), np.float32)
    for t in range(4):
        bqk[:, 0 + t] = bq[128 * t : 128 * (t + 1)]
        bqk[:, 4 + t] = bk[128 * t : 128 * (t + 1)]

    wp = np.ascontiguousarray(w_proj.reshape(H, 64, E).transpose(1, 0, 2))  # [64, H, E]
    bp = np.ascontiguousarray(b_proj.reshape(E, 1))

    xT = x.transpose(0, 2, 1)  # [B, 64, S]
    xT_dup = np.ascontiguousarray(np.concatenate([xT, xT], axis=1))  # [B, 128, S]

    wq_dup = np.ascontiguousarray(wq_dup)
    wk_dup = np.ascontiguousarray(wk_dup)
    wv_dup = np.ascontiguousarray(wv_dup)
    if variant[2] == "f32r":  # qkv matmul operands
        xT_dup = _round_f32r(xT_dup)
        wq_dup, wk_dup, wv_dup = map(_round_f32r, (wq_dup, wk_dup, wv_dup))
    if variant[3] == "f32r":  # proj stationary
        wp = _round_f32r(wp)

    shared = {
        "wq": wq_dup, "wk": wk_dup, "wv": wv_dup,
        "bqk": bqk, "bv": np.ascontiguousarray(bv), "wp": wp, "bp": bp,
    }
    in_maps = []
    for c in range(n_cores):
        m = dict(shared)
        m["xT"] = np.ascontiguousarray(xT_dup[c * bb : (c + 1) * bb])
        in_maps.append(m)
    return in_maps


_CACHE = {}


def run(inputs, trace=False, variant=("f32r", "f32r", "f32r", "f32r")):
    from concourse.bass_utils import run_bass_kernel_spmd

    key = variant
    if key not in _CACHE:
        dt_e, dt_pv, dt_qkv, dt_proj = variant
        _CACHE[key] = build_nc(dt_e=dt_e, dt_pv=dt_pv, dt_qkv=dt_qkv, dt_proj=dt_proj)
    nc = _CACHE[key]
    in_maps = prep_inputs(**inputs, variant=variant)
    res = run_bass_kernel_spmd(nc, in_maps, core_ids=list(range(N_CORES)), trace=trace)
    bb = B // N_CORES
    y = np.concatenate(
        [res.results[c]["yT"].transpose(0, 2, 1) for c in range(N_CORES)], axis=0
    )
    return np.ascontiguousarray(y), res


def kernel(x, w_qkv, b_qkv, w_proj, b_proj):
    y, _ = run(dict(x=x, w_qkv=w_qkv, b_qkv=b_qkv, w_proj=w_proj, b_proj=b_proj))
    return y

